# revision 1
# baseline (speedup 1.0000x reference)
"""AWQ 4-bit quantized linear (nn_AWQLinear) on 8 Trainium2 NeuronCores.

out[b,s,o] = fp16(sum_k x[b,s,k] * w[o,k]) + bias[o]
w[o,k] = (q[o,k] - z[o,k//128]) * s[o,k//128],  q packed 8 nibbles / int32.

Sharding: column-parallel (per spec hint). qweight/qzeros/scales/bias are
split along O=11008 into 8 shards of 1376; x is replicated; per-core
[4096, 1376] outputs are concatenated on host.

Host prep is layout-only (permute/replicate/bit-view; the only value math
is unpacking the 32 zero nibbles per shard row): qweight is viewed as
uint16 and its halfword-columns repeated 4x per k-group ("qwrep") so one
DMA-transpose per group lands a partition-replicated tile; x is K-permuted
(partition p = t*32+c holds k = 4c+t within each 128-group) and
pre-transposed to [K, M] so xT tiles load as plain contiguous DMAs;
scales/zeros ship as partition-replicated fp16 rows ("szrep").

Per-core kernel:
  1. Dequant (one-time, ~0.1 ms, PE-free), k-major, two groups at a time:
     two DMA-transposes -> rt [128, 2x1376] uint16 where partition
     p = t*32+c holds halfword c of both groups; DVE bitwise_and with a
     per-partition mask 0xF<<4t (nib*2^4t is exact in fp16: 4-bit
     mantissa); ScalarE activation(Copy, scale=2^-4t per partition)
     converts uint16->fp16 and applies the shift; two DVE tensor_tensor
     ops apply (nib - z) * s against partition-replicated z/s rows.
     Weight pairs are write-once tiles (no WAR hazard with PE reads);
     the full fp16 W^T (11 MB) stays resident in SBUF.
  2. Matmul: psum [m=128, o<=512] accumulates 32 k-matmuls
     (lhsT = xT tile slice, rhs = W^T group tile, 1 col/cycle fp16);
     m-block 0 runs k-major across 6-8 psum banks so the PE consumes each
     dequant pair as it lands (keeps HAM warm); later m-blocks run
     ms-outer with 3 banks and ob-inner so consecutive matmuls share the
     stationary operand. Epilogue: single DVE tensor_tensor adds the
     partition-replicated bias while copying PSUM->SBUF fp16; outputs DMA
     out on the GPSIMD queue (keeps the Sync queue free for xT loads).
"""

import sys

sys.path.insert(0, "/opt/trn_rl_repo")

import numpy as np

import concourse.bass as bass
import concourse.tile as tile
from concourse import bacc, mybir
from concourse import bass_utils

P = 128
N_CORES = 8
O_FULL = 11008
O_SHARD = O_FULL // N_CORES  # 1376
K = 4096
G = 32  # k-groups of 128
M = 4096  # tokens = 2*2048
M_TILE = 512
O_TILES = [(0, 512), (512, 512), (1024, O_SHARD - 1024)]  # (offset, width)
KMAJOR_MB0 = False  # k-major mb0 measured neutral once dequant got fast

f16 = mybir.dt.float16
u16 = mybir.dt.uint16
f32 = mybir.dt.float32


def build(n_mblocks=M // M_TILE, repeat=1):
    nc = bacc.Bacc("TRN2", target_bir_lowering=False, debug=False, num_devices=N_CORES)

    x_ap = nc.dram_tensor("xT", (K, M), f16, kind="ExternalInput").ap()
    qw_ap = nc.dram_tensor("qwrep", (O_SHARD, K), u16, kind="ExternalInput").ap()
    szr_ap = nc.dram_tensor("szrep", (G // 2, P, 2, 2 * O_SHARD), f16, kind="ExternalInput").ap()
    bias_ap = nc.dram_tensor("bias", (1, O_SHARD), f16, kind="ExternalInput").ap()
    msk_ap = nc.dram_tensor("msk", (P, 2 * O_SHARD), u16, kind="ExternalInput").ap()
    psc_ap = nc.dram_tensor("psc", (P, 1), f32, kind="ExternalInput").ap()
    out_ap = nc.dram_tensor(
        "out", (n_mblocks * M_TILE, O_SHARD), f16, kind="ExternalOutput"
    ).ap()

    with tile.TileContext(nc) as tc:
      for _rep in range(repeat):
        with (
            tc.tile_pool(name="const", bufs=1) as const,
            tc.tile_pool(name="wt", bufs=G // 2) as wt_pool,
            tc.tile_pool(name="deq", bufs=3) as deq,
            tc.tile_pool(name="xt", bufs=48) as xt_pool,
            tc.tile_pool(name="outp", bufs=6) as outp,
            tc.tile_pool(name="psum", bufs=8, space="PSUM") as psum,
        ):
            msk_sb = const.tile([P, 2 * O_SHARD], u16)
            nc.sync.dma_start(out=msk_sb, in_=msk_ap)
            psc_sb = const.tile([P, 1], f32)
            nc.sync.dma_start(out=psc_sb, in_=psc_ap)
            bias_rep = const.tile([P, O_SHARD], f16)
            nc.gpsimd.dma_start(
                out=bias_rep,
                in_=bass.AP(
                    tensor=bias_ap.tensor,
                    offset=bias_ap.offset,
                    ap=[[0, P], [1, O_SHARD]],
                ),
            )

            # per-group-pair weight tiles: write-once, so dequant never
            # carries a whole-tile WAR hazard against PE reads
            WTp = [
                wt_pool.tile([P, 2, O_SHARD], f16, tag="wt", name=f"wt{_rep}_{q}")
                for q in range(G // 2)
            ]

            # ---- dequant ----
            # qwrep column g*128 + t*32 + c = halfword g*32+c of qweight, so a
            # single DMA-transpose lands the 4x-partition-replicated tile.
            # Groups processed in pairs (ops on [P, 2752]) to amortize per-op
            # overhead; the two rt transposes ride different HWDGE queues.
            xts0 = []
            for q in range(G // 2):
                ga, gb = 2 * q, 2 * q + 1
                rt = deq.tile([P, 2, O_SHARD], u16, tag="rt", bufs=5)
                nc.sync.dma_start_transpose(
                    rt[:, 0, :], qw_ap[:, ga * P : (ga + 1) * P]
                )
                nc.sync.dma_start_transpose(
                    rt[:, 1, :], qw_ap[:, gb * P : (gb + 1) * P]
                )
                szr = deq.tile([P, 2, 2, O_SHARD], f16, tag="szr", bufs=3)
                nc.gpsimd.dma_start(out=szr, in_=szr_ap[q])
                wg = WTp[q]
                rtf = rt.rearrange("p a o -> p (a o)")
                wgf = wg.rearrange("p a o -> p (a o)")
                # nib*2^4t = hw & (0xF << 4t); exact in fp16 (4-bit mantissa)
                nc.vector.tensor_tensor(
                    out=rtf,
                    in0=rtf,
                    in1=msk_sb,
                    op=mybir.AluOpType.bitwise_and,
                )
                # uint16 -> fp16 with per-partition 2^-4t scale (the shift)
                nc.scalar.activation(
                    out=wgf,
                    in_=rtf,
                    func=mybir.ActivationFunctionType.Copy,
                    scale=psc_sb,
                )
                nc.vector.tensor_tensor(
                    out=wgf,
                    in0=wgf,
                    in1=szr.rearrange("p a b o -> p (a b o)")[:, O_SHARD * 2 :],
                    op=mybir.AluOpType.subtract,
                )
                nc.vector.tensor_tensor(
                    out=wgf,
                    in0=wgf,
                    in1=szr.rearrange("p a b o -> p (a b o)")[:, : O_SHARD * 2],
                    op=mybir.AluOpType.mult,
                )

            for g in range(G):
                xtile = xt_pool.tile([P, M_TILE], f16, tag="xt", name="xt")
                nc.sync.dma_start(
                    out=xtile, in_=x_ap[g * P : (g + 1) * P, 0:M_TILE]
                )
                xts0.append(xtile)

            # ---- matmul ----
            def finish_group(ps, mb, ms, o0, ow):
                ot = outp.tile([P, 512], f16, tag="ot", name="ot")
                # psum + bias -> sbuf fp16, one DVE op (ACT queue stays free)
                nc.vector.tensor_tensor(
                    out=ot[:, :ow],
                    in0=ps,
                    in1=bias_rep[:, o0 : o0 + ow],
                    op=mybir.AluOpType.add,
                )
                m0 = mb * M_TILE + ms * P
                nc.gpsimd.dma_start(
                    out=out_ap[m0 : m0 + P, o0 : o0 + ow], in_=ot[:, :ow]
                )

            def kmajor_pass(mb, xts, groups):
                # one psum bank per (o-tile, ms); k-major so PE consumes
                # each dequant group as soon as it lands
                pss = []
                for _ in groups:
                    pst = psum.tile([P, 512], f32, tag="ps", name="ps")
                    pss.append(pst)
                for kt in range(G):
                    for i, (o0, ow, ms) in enumerate(groups):
                        nc.tensor.matmul(
                            pss[i][:, :ow],
                            lhsT=xts[kt][:, ms * P : (ms + 1) * P],
                            rhs=WTp[kt // 2][:, kt % 2, o0 : o0 + ow],
                            start=(kt == 0),
                            stop=(kt == G - 1),
                        )
                for i, (o0, ow, ms) in enumerate(groups):
                    finish_group(pss[i][:, :ow], mb, ms, o0, ow)

            for mb in range(n_mblocks):
                if mb == 0:
                    xts = xts0
                else:
                    xts = []
                    for kt in range(G):
                        xtile = xt_pool.tile([P, M_TILE], f16, tag="xt", name="xt")
                        nc.sync.dma_start(
                            out=xtile,
                            in_=x_ap[
                                kt * P : (kt + 1) * P,
                                mb * M_TILE : (mb + 1) * M_TILE,
                            ],
                        )
                        xts.append(xtile)
                if mb == 0 and KMAJOR_MB0:
                    # dequant still streaming: spread PE work k-major;
                    # ms-outer so consecutive MMs share the stationary xT
                    kmajor_pass(0, xts, [(o0, ow, ms) for ms in (0, 1)
                                         for (o0, ow) in O_TILES])
                    kmajor_pass(0, xts, [(o0, ow, ms) for ms in (2, 3)
                                         for (o0, ow) in O_TILES])
                    continue
                for ms in range(M_TILE // P):
                    # 3 psum banks, kt-major, ob-inner: one LDWEIGHTS feeds
                    # the 3 o-tiles (redundant loads removed by ldw-opt)
                    pss = []
                    for _ in O_TILES:
                        pss.append(psum.tile([P, 512], f32, tag="ps", name="ps"))
                    for kt in range(G):
                        for i, (o0, ow) in enumerate(O_TILES):
                            nc.tensor.matmul(
                                pss[i][:, :ow],
                                lhsT=xts[kt][:, ms * P : (ms + 1) * P],
                                rhs=WTp[kt // 2][:, kt % 2, o0 : o0 + ow],
                                start=(kt == 0),
                                stop=(kt == G - 1),
                            )
                    for i, (o0, ow) in enumerate(O_TILES):
                        finish_group(pss[i][:, :ow], mb, ms, o0, ow)

    nc.compile()
    return nc


def _unpack_nib(a):
    shifts = (np.arange(8, dtype=np.int32) * 4).reshape(1, 1, 8)
    nib = (a[..., None] >> shifts) & 0xF
    return nib.reshape(a.shape[0], a.shape[1] * 8)


def make_in_maps(x, qweight, qzeros, scales, bias):
    # Permute K within each 128-group to match the device k-partition layout
    # (device partition p = t*32 + c holds original k = g*128 + 4*c + t) and
    # transpose to [K, M] so xT tiles load with plain contiguous DMAs.
    x_flat = np.ascontiguousarray(
        x.reshape(M, G, 32, 4).transpose(1, 3, 2, 0).reshape(K, M)
    )
    msk = np.broadcast_to(
        (np.uint16(0xF) << (4 * (np.arange(P) // 32))).astype(np.uint16).reshape(P, 1),
        (P, 2 * O_SHARD),
    )
    msk = np.ascontiguousarray(msk)
    psc = (2.0 ** (-4.0 * (np.arange(P) // 32))).astype(np.float32).reshape(P, 1)
    in_maps = []
    for i in range(N_CORES):
        sl = slice(i * O_SHARD, (i + 1) * O_SHARD)
        qw16 = np.ascontiguousarray(qweight[sl]).view(np.uint16)
        j = np.arange(K)
        qwrep = np.ascontiguousarray(qw16[:, (j // 128) * 32 + (j % 32)])
        z = _unpack_nib(np.ascontiguousarray(qzeros[sl]))[:, :G].astype(np.float16)
        s = scales[sl, :G]
        szt = np.stack([s.T, z.T], axis=1).astype(np.float16)  # [G, 2, O]
        # pair layout: [q, p, {s,z}, 2*O] with the two groups' rows concat
        szp = szt.reshape(G // 2, 2, 2, O_SHARD).transpose(0, 2, 1, 3)
        szp = szp.reshape(G // 2, 1, 2, 2 * O_SHARD)
        szrep = np.ascontiguousarray(
            np.broadcast_to(szp, (G // 2, P, 2, 2 * O_SHARD))
        )
        b = np.ascontiguousarray(bias[sl]).reshape(1, O_SHARD)
        in_maps.append(
            {"xT": x_flat, "qwrep": qwrep, "szrep": szrep, "bias": b, "msk": msk,
             "psc": psc}
        )
    return in_maps


_NC = None


def kernel(x, qweight, qzeros, scales, bias):
    global _NC
    x = np.asarray(x)
    qweight = np.asarray(qweight)
    qzeros = np.asarray(qzeros)
    scales = np.asarray(scales)
    bias = np.asarray(bias)
    if _NC is None:
        _NC = build()
    in_maps = make_in_maps(x, qweight, qzeros, scales, bias)
    res = bass_utils.run_bass_kernel_spmd(_NC, in_maps, core_ids=list(range(N_CORES)))
    shards = [res.results[i]["out"] for i in range(N_CORES)]
    out = np.concatenate(shards, axis=1).reshape(2, 2048, O_FULL)
    return out.astype(np.float16)



# revision 2
# speedup vs baseline: 1.0535x; 1.0535x over previous
"""AWQ 4-bit quantized linear (nn_AWQLinear) on 8 Trainium2 NeuronCores.

out[b,s,o] = fp16(sum_k x[b,s,k] * w[o,k]) + bias[o]
w[o,k] = (q[o,k] - z[o,k//128]) * s[o,k//128],  q packed 8 nibbles / int32.

Sharding: column-parallel (per spec hint). qweight/qzeros/scales/bias are
split along O=11008 into 8 shards of 1376; x is replicated; per-core
[4096, 1376] outputs are concatenated on host.

v2 layout — quad-packed dequant (no replicated shipping):
  K is processed in 32 chunks of 128; chunk kt = (Q, t) with Q = kt//4 a
  "quad" of 4 consecutive k-groups and t = kt%4 a nibble index. Partition
  p = j*32 + c of chunk (Q, t) holds original k = (4Q+j)*128 + 4c + t, so
  ONE [128, 1376] u16 tile ("qwq", halfword c of group 4Q+j at column o,
  host-gathered, unique bytes only) serves all four nibble extractions:
    slice t:  and-mask 0xF<<4t (DVE tensor_scalar, per-partition scalar)
              -> STT (masked * 2^-4t) * s_b  (u16->f16 convert fused)
              -> TT  w -= zs_b               (zs = z*s, host-prepped)
  s_b/zs_b come from one DMA per quad that block-replicates 4 rows of
  scales 32x across partitions ([4g,2,1376] -> [128,2,1376], 0.7 MB);
  x is host-permuted to the chunk layout and pre-transposed to [K, M].
  Startup DMA fabric drops ~42MB -> ~13MB vs replicated shipping, so the
  PE can consume dequant quads nearly as fast as they land.

Matmul: psum [m=128, o<=512] accumulates 32 k-chunk matmuls (lhsT = xT
tile slice, rhs = W quad slice). mb0 runs k-major across 6 psum banks in
two passes so the PE consumes each quad as it lands; later m-blocks run
ms-outer with 3 banks and ob-inner so consecutive matmuls share the
stationary operand. Epilogue: single DVE tensor_tensor adds the
partition-replicated bias while copying PSUM->SBUF fp16; outputs ride
the scalar HWDGE queue.
"""

import sys

sys.path.insert(0, "/opt/trn_rl_repo")

import numpy as np

import concourse.bass as bass
import concourse.tile as tile
from concourse import bacc, mybir
from concourse import bass_utils

P = 128
N_CORES = 8
O_FULL = 11008
O_SHARD = O_FULL // N_CORES  # 1376
K = 4096
G = 32  # k-groups of 128
QUADS = G // 4  # 8
M = 4096  # tokens = 2*2048
M_TILE = 512
O_TILES = [(0, 512), (512, 512), (1024, O_SHARD - 1024)]  # (offset, width)
KMAJOR_MBS = 1  # leading m-blocks run k-major to ride the dequant wave
# engine for the zs-subtract of slice t: True -> gpsimd, False -> DVE
TT_ON_GPSIMD = (True, True, True, True)

f16 = mybir.dt.float16
u16 = mybir.dt.uint16
f32 = mybir.dt.float32


def build(n_mblocks=M // M_TILE, repeat=1):
    nc = bacc.Bacc("TRN2", target_bir_lowering=False, debug=False, num_devices=N_CORES)

    x_ap = nc.dram_tensor("xT", (K, M), f16, kind="ExternalInput").ap()
    qwq_ap = nc.dram_tensor("qwq", (QUADS, P, O_SHARD), u16, kind="ExternalInput").ap()
    szq_ap = nc.dram_tensor("szq", (QUADS, 4, 2, O_SHARD), f16, kind="ExternalInput").ap()
    bias_ap = nc.dram_tensor("bias", (1, O_SHARD), f16, kind="ExternalInput").ap()
    msk_ap = nc.dram_tensor("msk", (P, 4), u16, kind="ExternalInput").ap()
    psc_ap = nc.dram_tensor("psc", (P, 4), f32, kind="ExternalInput").ap()
    out_ap = nc.dram_tensor(
        "out", (n_mblocks * M_TILE, O_SHARD), f16, kind="ExternalOutput"
    ).ap()

    with tile.TileContext(nc) as tc:
      for _rep in range(repeat):
        with (
            tc.tile_pool(name="const", bufs=1) as const,
            tc.tile_pool(name="wt", bufs=QUADS) as wt_pool,
            tc.tile_pool(name="deq", bufs=3) as deq,
            tc.tile_pool(name="xt", bufs=48) as xt_pool,
            tc.tile_pool(name="outp", bufs=6) as outp,
            tc.tile_pool(name="psum", bufs=8, space="PSUM") as psum,
        ):
            msk_sb = const.tile([P, 4], u16)
            nc.sync.dma_start(out=msk_sb, in_=msk_ap)
            psc_sb = const.tile([P, 4], f32)
            nc.sync.dma_start(out=psc_sb, in_=psc_ap)
            bias_rep = const.tile([P, O_SHARD], f16)
            nc.scalar.dma_start(
                out=bias_rep,
                in_=bass.AP(
                    tensor=bias_ap.tensor,
                    offset=bias_ap.offset,
                    ap=[[0, P], [1, O_SHARD]],
                ),
            )

            # per-quad weight tiles: each [128, 4 t-slices, 1376]
            WQ = [
                wt_pool.tile([P, 4, O_SHARD], f16, tag="wt", name=f"wq{_rep}_{q}")
                for q in range(QUADS)
            ]

            # ---- dequant ----
            xts0 = []
            for q in range(QUADS):
                rt = deq.tile([P, O_SHARD], u16, tag="rt", bufs=3)
                nc.sync.dma_start(out=rt, in_=qwq_ap[q])
                szb = deq.tile([P, 2, O_SHARD], f16, tag="szb", bufs=3)
                nc.scalar.dma_start(
                    out=szb,
                    in_=bass.AP(
                        tensor=szq_ap.tensor,
                        offset=szq_ap.offset + q * 4 * 2 * O_SHARD,
                        ap=[[2 * O_SHARD, 4], [0, 32], [O_SHARD, 2], [1, O_SHARD]],
                    ),
                )
                for t in range(4):
                    na = deq.tile([P, O_SHARD], u16, tag="na", bufs=4)
                    nc.vector.tensor_scalar(
                        out=na,
                        in0=rt,
                        scalar1=msk_sb[:, t : t + 1],
                        scalar2=None,
                        op0=mybir.AluOpType.bitwise_and,
                    )
                    wslice = WQ[q][:, t, :]
                    nc.vector.scalar_tensor_tensor(
                        out=wslice,
                        in0=na,
                        scalar=psc_sb[:, t : t + 1],
                        in1=szb[:, 0, :],
                        op0=mybir.AluOpType.mult,
                        op1=mybir.AluOpType.mult,
                    )
                    eng = nc.gpsimd if TT_ON_GPSIMD[t] else nc.vector
                    eng.tensor_tensor(
                        out=wslice,
                        in0=wslice,
                        in1=szb[:, 1, :],
                        op=mybir.AluOpType.subtract,
                    )

            for g in range(G):
                xtile = xt_pool.tile([P, M_TILE], f16, tag="xt", name="xt")
                nc.sync.dma_start(
                    out=xtile, in_=x_ap[g * P : (g + 1) * P, 0:M_TILE]
                )
                xts0.append(xtile)

            # ---- matmul ----
            def finish_group(ps, mb, ms, o0, ow):
                ot = outp.tile([P, 512], f16, tag="ot", name="ot")
                nc.vector.tensor_tensor(
                    out=ot[:, :ow],
                    in0=ps,
                    in1=bias_rep[:, o0 : o0 + ow],
                    op=mybir.AluOpType.add,
                )
                m0 = mb * M_TILE + ms * P
                nc.scalar.dma_start(
                    out=out_ap[m0 : m0 + P, o0 : o0 + ow], in_=ot[:, :ow]
                )

            def kmajor_pass(mb, xts, groups):
                # one psum bank per (o-tile, ms); k-major so PE consumes
                # each dequant quad as soon as it lands
                pss = []
                for _ in groups:
                    pst = psum.tile([P, 512], f32, tag="ps", name="ps")
                    pss.append(pst)
                for kt in range(G):
                    for i, (o0, ow, ms) in enumerate(groups):
                        nc.tensor.matmul(
                            pss[i][:, :ow],
                            lhsT=xts[kt][:, ms * P : (ms + 1) * P],
                            rhs=WQ[kt // 4][:, kt % 4, o0 : o0 + ow],
                            start=(kt == 0),
                            stop=(kt == G - 1),
                        )
                for i, (o0, ow, ms) in enumerate(groups):
                    finish_group(pss[i][:, :ow], mb, ms, o0, ow)

            for mb in range(n_mblocks):
                if mb == 0:
                    xts = xts0
                else:
                    xts = []
                    for kt in range(G):
                        xtile = xt_pool.tile([P, M_TILE], f16, tag="xt", name="xt")
                        nc.sync.dma_start(
                            out=xtile,
                            in_=x_ap[
                                kt * P : (kt + 1) * P,
                                mb * M_TILE : (mb + 1) * M_TILE,
                            ],
                        )
                        xts.append(xtile)
                if mb < KMAJOR_MBS:
                    # dequant still streaming: spread PE work k-major;
                    # ms-outer so consecutive MMs share the stationary xT
                    kmajor_pass(mb, xts, [(o0, ow, ms) for ms in (0, 1)
                                          for (o0, ow) in O_TILES])
                    kmajor_pass(mb, xts, [(o0, ow, ms) for ms in (2, 3)
                                          for (o0, ow) in O_TILES])
                    continue
                for ms in range(M_TILE // P):
                    # 3 psum banks, kt-major, ob-inner: one LDWEIGHTS feeds
                    # the 3 o-tiles (redundant loads removed by ldw-opt)
                    pss = []
                    for _ in O_TILES:
                        pss.append(psum.tile([P, 512], f32, tag="ps", name="ps"))
                    for kt in range(G):
                        for i, (o0, ow) in enumerate(O_TILES):
                            nc.tensor.matmul(
                                pss[i][:, :ow],
                                lhsT=xts[kt][:, ms * P : (ms + 1) * P],
                                rhs=WQ[kt // 4][:, kt % 4, o0 : o0 + ow],
                                start=(kt == 0),
                                stop=(kt == G - 1),
                            )
                    for i, (o0, ow) in enumerate(O_TILES):
                        finish_group(pss[i][:, :ow], mb, ms, o0, ow)

    nc.compile()
    return nc


def _unpack_nib(a):
    shifts = (np.arange(8, dtype=np.int32) * 4).reshape(1, 1, 8)
    nib = (a[..., None] >> shifts) & 0xF
    return nib.reshape(a.shape[0], a.shape[1] * 8)


def make_in_maps(x, qweight, qzeros, scales, bias):
    # Chunk kt=(Q,t): partition p = j*32+c holds original k = (4Q+j)*128+4c+t.
    # Permute K accordingly and transpose to [K, M] so xT tiles load with
    # plain contiguous DMAs.
    x_flat = np.ascontiguousarray(
        x.reshape(M, QUADS, 4, 32, 4)      # [m, Q, j, c, t]
        .transpose(1, 4, 2, 3, 0)           # [Q, t, j, c, m]
        .reshape(K, M)
    )
    msk = np.broadcast_to(
        (np.uint16(0xF) << (4 * np.arange(4))).astype(np.uint16).reshape(1, 4),
        (P, 4),
    )
    msk = np.ascontiguousarray(msk)
    psc = np.ascontiguousarray(np.broadcast_to(
        (2.0 ** (-4.0 * np.arange(4))).astype(np.float32).reshape(1, 4), (P, 4)
    ))
    in_maps = []
    for i in range(N_CORES):
        sl = slice(i * O_SHARD, (i + 1) * O_SHARD)
        qw16 = np.ascontiguousarray(qweight[sl]).view(np.uint16)  # [O, 1024]
        # qwq[Q, j*32+c, o] = halfword (4Q+j)*32+c of row o
        qwq = np.ascontiguousarray(
            qw16.T.reshape(QUADS, 4 * 32, O_SHARD)
        )
        z = _unpack_nib(np.ascontiguousarray(qzeros[sl]))[:, :G].astype(np.float32)
        s = scales[sl, :G].astype(np.float32)
        zs = (z * s).astype(np.float16)  # [O, G]
        st = s.astype(np.float16)
        # szq[Q, g_in_quad, {s, zs}, o]
        szq = np.stack([st.T.reshape(G, O_SHARD), zs.T.reshape(G, O_SHARD)],
                       axis=1)               # [G, 2, O]
        szq = np.ascontiguousarray(szq.reshape(QUADS, 4, 2, O_SHARD))
        b = np.ascontiguousarray(bias[sl]).reshape(1, O_SHARD)
        in_maps.append(
            {"xT": x_flat, "qwq": qwq, "szq": szq, "bias": b, "msk": msk,
             "psc": psc}
        )
    return in_maps


_NC = None


def kernel(x, qweight, qzeros, scales, bias):
    global _NC
    x = np.asarray(x)
    qweight = np.asarray(qweight)
    qzeros = np.asarray(qzeros)
    scales = np.asarray(scales)
    bias = np.asarray(bias)
    if _NC is None:
        _NC = build()
    in_maps = make_in_maps(x, qweight, qzeros, scales, bias)
    res = bass_utils.run_bass_kernel_spmd(_NC, in_maps, core_ids=list(range(N_CORES)))
    shards = [res.results[i]["out"] for i in range(N_CORES)]
    out = np.concatenate(shards, axis=1).reshape(2, 2048, O_FULL)
    return out.astype(np.float16)


# revision 3
# speedup vs baseline: 1.1067x; 1.0504x over previous
"""AWQ 4-bit quantized linear (nn_AWQLinear) on 8 Trainium2 NeuronCores.

out[b,s,o] = fp16(sum_k x[b,s,k] * w[o,k]) + bias[o]
w[o,k] = (q[o,k] - z[o,k//128]) * s[o,k//128],  q packed 8 nibbles / int32.

Sharding: column-parallel (per spec hint). qweight/qzeros/scales/bias are
split along O=11008 into 8 shards of 1376; x is replicated; per-core
[4096, 1376] outputs are concatenated on host.

v3 layout — quad-packed dequant, engine-balanced:
  K is processed in 32 chunks of 128; chunk kt = (Q, t) with Q = kt//4 a
  "quad" of 4 consecutive k-groups and t = kt%4 a nibble index. Partition
  p = j*32 + c of chunk (Q, t) holds original k = (4Q+j)*128 + 4c + t, so
  ONE [128, 1376] u16 tile ("qwq", halfword c of group 4Q+j at column o,
  host-gathered, unique bytes only) serves all four nibble extractions:
    slice t:  u32-bitcast AND with packed mask (DVE tensor_scalar, halves
              the column count -> 2x tier)
              -> ScalarE activation Copy(scale=2^-4t): u16 -> f16 nibble
              -> TT w = nib * s_b ; TT w -= zs_b  (f16, 2x tier;
                 slices 0-2 on DVE, slice 3 on gpsimd for balance)
  s_b/zs_b come from one DMA per quad that block-replicates 4 rows of
  scales 32x across partitions ([4g,2,1376] -> [128,2,1376], 0.7 MB);
  zs = z*s host-prepped. x is host-permuted to the chunk layout and
  pre-transposed to [K, M]. rt pool is 8 deep so all qweight DMAs issue
  up front (sync queue never blocks on pool recycling before x loads).

Matmul: psum [m=128, o<=512] accumulates 32 k-chunk matmuls (lhsT = xT
tile slice, rhs = W quad slice). mb0 runs k-major: pass A fills all 8
psum banks (ms0,1 x 3 o-tiles + ms2 x 2) consuming ~6.3us/quad to match
dequant production; pass B covers the remaining 4 tiles. Later m-blocks
run ms-outer with 3 banks and ob-inner so consecutive matmuls share the
stationary operand. Epilogue: single DVE tensor_tensor adds the
partition-replicated bias while copying PSUM->SBUF fp16; outputs ride
the scalar HWDGE queue (last m-block alternates scalar/sync to halve
the final drain).
"""

import sys

sys.path.insert(0, "/opt/trn_rl_repo")

import numpy as np

import concourse.bass as bass
import concourse.tile as tile
from concourse import bacc, mybir
from concourse import bass_utils

P = 128
N_CORES = 8
O_FULL = 11008
O_SHARD = O_FULL // N_CORES  # 1376
K = 4096
G = 32  # k-groups of 128
QUADS = G // 4  # 8
M = 4096  # tokens = 2*2048
M_TILE = 512
O_TILES = [(0, 512), (512, 512), (1024, O_SHARD - 1024)]  # (offset, width)

f16 = mybir.dt.float16
u16 = mybir.dt.uint16
u32 = mybir.dt.uint32
f32 = mybir.dt.float32


def build(n_mblocks=M // M_TILE, repeat=1):
    nc = bacc.Bacc("TRN2", target_bir_lowering=False, debug=False, num_devices=N_CORES)

    x_ap = nc.dram_tensor("xT", (K, M), f16, kind="ExternalInput").ap()
    qwq_ap = nc.dram_tensor("qwq", (QUADS, P, O_SHARD), u16, kind="ExternalInput").ap()
    szq_ap = nc.dram_tensor("szq", (QUADS, 4, 2, O_SHARD), f16, kind="ExternalInput").ap()
    bias_ap = nc.dram_tensor("bias", (1, O_SHARD), f16, kind="ExternalInput").ap()
    msk_ap = nc.dram_tensor("msk", (P, 4), u32, kind="ExternalInput").ap()
    out_ap = nc.dram_tensor(
        "out", (n_mblocks * M_TILE, O_SHARD), f16, kind="ExternalOutput"
    ).ap()

    with tile.TileContext(nc) as tc:
      for _rep in range(repeat):
        with (
            tc.tile_pool(name="const", bufs=1) as const,
            tc.tile_pool(name="wt", bufs=QUADS) as wt_pool,
            tc.tile_pool(name="deq", bufs=3) as deq,
            tc.tile_pool(name="xt", bufs=44) as xt_pool,
            tc.tile_pool(name="outp", bufs=6) as outp,
            tc.tile_pool(name="psum", bufs=8, space="PSUM") as psum,
        ):
            msk_sb = const.tile([P, 4], u32)
            nc.sync.dma_start(out=msk_sb, in_=msk_ap)
            bias_rep = const.tile([P, O_SHARD], f16)
            nc.gpsimd.dma_start(
                out=bias_rep,
                in_=bass.AP(
                    tensor=bias_ap.tensor,
                    offset=bias_ap.offset,
                    ap=[[0, P], [1, O_SHARD]],
                ),
            )

            # per-quad weight tiles: each [128, 4 t-slices, 1376]
            WQ = [
                wt_pool.tile([P, 4, O_SHARD], f16, tag="wt", name=f"wq{_rep}_{q}")
                for q in range(QUADS)
            ]

            # ---- dequant ----
            xts0 = []
            for q in range(QUADS):
                rt = deq.tile([P, O_SHARD], u16, tag="rt", bufs=QUADS)
                nc.sync.dma_start(out=rt, in_=qwq_ap[q])
                szb = deq.tile([P, 2, O_SHARD], f16, tag="szb", bufs=4)
                nc.scalar.dma_start(
                    out=szb,
                    in_=bass.AP(
                        tensor=szq_ap.tensor,
                        offset=szq_ap.offset + q * 4 * 2 * O_SHARD,
                        ap=[[2 * O_SHARD, 4], [0, 32], [O_SHARD, 2], [1, O_SHARD]],
                    ),
                )
                for t in range(4):
                    na = deq.tile([P, O_SHARD], u16, tag="na", bufs=4)
                    nc.vector.tensor_scalar(
                        out=na.bitcast(u32),
                        in0=rt.bitcast(u32),
                        scalar1=msk_sb[:, t : t + 1],
                        scalar2=None,
                        op0=mybir.AluOpType.bitwise_and,
                    )
                    nf = deq.tile([P, O_SHARD], f16, tag="nf", bufs=4)
                    nc.scalar.activation(
                        out=nf,
                        in_=na,
                        func=mybir.ActivationFunctionType.Copy,
                        scale=float(2.0 ** (-4 * t)),
                    )
                    wslice = WQ[q][:, t, :]
                    eng = nc.gpsimd if t == 3 else nc.vector
                    eng.tensor_tensor(
                        out=wslice,
                        in0=nf,
                        in1=szb[:, 0, :],
                        op=mybir.AluOpType.mult,
                    )
                    eng.tensor_tensor(
                        out=wslice,
                        in0=wslice,
                        in1=szb[:, 1, :],
                        op=mybir.AluOpType.subtract,
                    )

            for g in range(G):
                xtile = xt_pool.tile([P, M_TILE], f16, tag="xt", name="xt")
                nc.sync.dma_start(
                    out=xtile, in_=x_ap[g * P : (g + 1) * P, 0:M_TILE]
                )
                xts0.append(xtile)

            # ---- matmul ----
            def finish_group(ps, mb, ms, o0, ow, qi=0):
                ot = outp.tile([P, 512], f16, tag="ot", name="ot")
                nc.vector.tensor_tensor(
                    out=ot[:, :ow],
                    in0=ps,
                    in1=bias_rep[:, o0 : o0 + ow],
                    op=mybir.AluOpType.add,
                )
                m0 = mb * M_TILE + ms * P
                qeng = nc.sync if (mb == n_mblocks - 1 and qi % 2) else nc.scalar
                qeng.dma_start(
                    out=out_ap[m0 : m0 + P, o0 : o0 + ow], in_=ot[:, :ow]
                )

            def kmajor_pass(mb, xts, groups):
                # one psum bank per (o-tile, ms); k-major so PE consumes
                # each dequant quad as soon as it lands
                pss = []
                for _ in groups:
                    pst = psum.tile([P, 512], f32, tag="ps", name="ps")
                    pss.append(pst)
                for kt in range(G):
                    for i, (o0, ow, ms) in enumerate(groups):
                        nc.tensor.matmul(
                            pss[i][:, :ow],
                            lhsT=xts[kt][:, ms * P : (ms + 1) * P],
                            rhs=WQ[kt // 4][:, kt % 4, o0 : o0 + ow],
                            start=(kt == 0),
                            stop=(kt == G - 1),
                        )
                for i, (o0, ow, ms) in enumerate(groups):
                    finish_group(pss[i][:, :ow], mb, ms, o0, ow)

            for mb in range(n_mblocks):
                if mb == 0:
                    xts = xts0
                else:
                    xts = []
                    for kt in range(G):
                        xtile = xt_pool.tile([P, M_TILE], f16, tag="xt", name="xt")
                        nc.sync.dma_start(
                            out=xtile,
                            in_=x_ap[
                                kt * P : (kt + 1) * P,
                                mb * M_TILE : (mb + 1) * M_TILE,
                            ],
                        )
                        xts.append(xtile)
                if mb == 0:
                    # dequant still streaming: pass A fills all 8 psum banks
                    # so the PE consumes each quad as fast as it lands
                    kmajor_pass(0, xts, [(o0, ow, ms) for ms in (0, 1)
                                         for (o0, ow) in O_TILES]
                                        + [(0, 512, 2), (512, 512, 2)])
                    kmajor_pass(0, xts, [(1024, O_SHARD - 1024, 2)]
                                        + [(o0, ow, 3) for (o0, ow) in O_TILES])
                    continue
                for ms in range(M_TILE // P):
                    # 3 psum banks, kt-major, ob-inner: one LDWEIGHTS feeds
                    # the 3 o-tiles (redundant loads removed by ldw-opt)
                    pss = []
                    for _ in O_TILES:
                        pss.append(psum.tile([P, 512], f32, tag="ps", name="ps"))
                    for kt in range(G):
                        for i, (o0, ow) in enumerate(O_TILES):
                            nc.tensor.matmul(
                                pss[i][:, :ow],
                                lhsT=xts[kt][:, ms * P : (ms + 1) * P],
                                rhs=WQ[kt // 4][:, kt % 4, o0 : o0 + ow],
                                start=(kt == 0),
                                stop=(kt == G - 1),
                            )
                    for i, (o0, ow) in enumerate(O_TILES):
                        finish_group(pss[i][:, :ow], mb, ms, o0, ow, qi=i + ms)

    nc.compile()
    return nc


def _unpack_nib(a):
    shifts = (np.arange(8, dtype=np.int32) * 4).reshape(1, 1, 8)
    nib = (a[..., None] >> shifts) & 0xF
    return nib.reshape(a.shape[0], a.shape[1] * 8)


def make_in_maps(x, qweight, qzeros, scales, bias):
    # Chunk kt=(Q,t): partition p = j*32+c holds original k = (4Q+j)*128+4c+t.
    # Permute K accordingly and transpose to [K, M] so xT tiles load with
    # plain contiguous DMAs.
    x_flat = np.ascontiguousarray(
        x.reshape(M, QUADS, 4, 32, 4)      # [m, Q, j, c, t]
        .transpose(1, 4, 2, 3, 0)           # [Q, t, j, c, m]
        .reshape(K, M)
    )
    mskv = (np.uint32(0xF) << (4 * np.arange(4, dtype=np.uint32))).astype(np.uint32)
    mskv = mskv | (mskv << np.uint32(16))   # pack the mask into both u16 halves
    msk = np.ascontiguousarray(np.broadcast_to(mskv.reshape(1, 4), (P, 4)))
    in_maps = []
    for i in range(N_CORES):
        sl = slice(i * O_SHARD, (i + 1) * O_SHARD)
        qw16 = np.ascontiguousarray(qweight[sl]).view(np.uint16)  # [O, 1024]
        # qwq[Q, j*32+c, o] = halfword (4Q+j)*32+c of row o
        qwq = np.ascontiguousarray(
            qw16.T.reshape(QUADS, 4 * 32, O_SHARD)
        )
        z = _unpack_nib(np.ascontiguousarray(qzeros[sl]))[:, :G].astype(np.float32)
        s = scales[sl, :G].astype(np.float32)
        zs = (z * s).astype(np.float16)  # [O, G]
        st = s.astype(np.float16)
        # szq[Q, g_in_quad, {s, zs}, o]
        szq = np.stack([st.T.reshape(G, O_SHARD), zs.T.reshape(G, O_SHARD)],
                       axis=1)               # [G, 2, O]
        szq = np.ascontiguousarray(szq.reshape(QUADS, 4, 2, O_SHARD))
        b = np.ascontiguousarray(bias[sl]).reshape(1, O_SHARD)
        in_maps.append(
            {"xT": x_flat, "qwq": qwq, "szq": szq, "bias": b, "msk": msk}
        )
    return in_maps


_NC = None


def kernel(x, qweight, qzeros, scales, bias):
    global _NC
    x = np.asarray(x)
    qweight = np.asarray(qweight)
    qzeros = np.asarray(qzeros)
    scales = np.asarray(scales)
    bias = np.asarray(bias)
    if _NC is None:
        _NC = build()
    in_maps = make_in_maps(x, qweight, qzeros, scales, bias)
    res = bass_utils.run_bass_kernel_spmd(_NC, in_maps, core_ids=list(range(N_CORES)))
    shards = [res.results[i]["out"] for i in range(N_CORES)]
    out = np.concatenate(shards, axis=1).reshape(2, 2048, O_FULL)
    return out.astype(np.float16)


# revision 4
# speedup vs baseline: 1.1186x; 1.0107x over previous
"""AWQ 4-bit quantized linear (nn_AWQLinear) on 8 Trainium2 NeuronCores.

out[b,s,o] = fp16(sum_k x[b,s,k] * w[o,k]) + bias[o]
w[o,k] = (q[o,k] - z[o,k//128]) * s[o,k//128],  q packed 8 nibbles / int32.

Sharding: column-parallel (per spec hint). qweight/qzeros/scales/bias are
split along O=11008 into 8 shards of 1376; x is replicated; per-core
[4096, 1376] outputs are concatenated on host.

v4 layout — quad-packed dequant, per-slice weight tiles:
  K is processed in 32 chunks of 128; chunk kt = (Q, t) with Q = kt//4 a
  "quad" of 4 consecutive k-groups and t = kt%4 a nibble index. Partition
  p = j*32 + c of chunk (Q, t) holds original k = (4Q+j)*128 + 4c + t, so
  ONE [128, 1376] u16 tile ("qwq", halfword c of group 4Q+j at column o,
  host-gathered, unique bytes only) serves all four nibble extractions:
    slice t:  u32-bitcast AND with packed mask (DVE tensor_scalar, halves
              the column count; mask tiles built by memset, no DMA)
              -> ScalarE activation Copy(scale=2^-4t): u16 -> f16 nibble
              -> TT w = nib * s_b ; TT w -= zs_b  (f16, 2x tier;
                 slices 0-2 on DVE, slice 3 on gpsimd for balance)
  Each W slice is its OWN tile so a chunk's matmuls depend only on that
  slice's two writers (no whole-quad false dependency). s_b/zs_b arrive
  as separate per-quad DMAs that block-replicate 4 scale rows 32x across
  partitions ([4g,1376] -> [128,1376], 0.35 MB each, 6-deep pools so the
  x-tile stream can't starve them); zs = z*s host-prepped. x is
  host-permuted to the chunk layout and pre-transposed to [K, M].

Matmul: psum [m=128, o<=512] accumulates 32 k-chunk matmuls (lhsT = xT
tile slice, rhs = W chunk slice). mb0 runs k-major: pass A fills all 8
psum banks (ms0,1 x 3 o-tiles + ms2 x 2) consuming ~6.3us/quad to ride
the dequant wave; pass B covers the remaining 4 tiles. Later m-blocks
run ms-outer with 3 banks and ob-inner so consecutive matmuls share the
stationary operand. Epilogue: single DVE tensor_tensor adds the
partition-replicated bias while copying PSUM->SBUF fp16; outputs ride
the scalar HWDGE queue (last m-block alternates scalar/sync to halve
the final drain).
"""

import sys

sys.path.insert(0, "/opt/trn_rl_repo")

import numpy as np

import concourse.bass as bass
import concourse.tile as tile
from concourse import bacc, mybir
from concourse import bass_utils

P = 128
N_CORES = 8
O_FULL = 11008
O_SHARD = O_FULL // N_CORES  # 1376
K = 4096
G = 32  # k-groups of 128
QUADS = G // 4  # 8
M = 4096  # tokens = 2*2048
M_TILE = 512
O_TILES = [(0, 512), (512, 512), (1024, O_SHARD - 1024)]  # (offset, width)

f16 = mybir.dt.float16
u16 = mybir.dt.uint16
u32 = mybir.dt.uint32
f32 = mybir.dt.float32


def build(n_mblocks=M // M_TILE, repeat=1):
    nc = bacc.Bacc("TRN2", target_bir_lowering=False, debug=False, num_devices=N_CORES)

    x_ap = nc.dram_tensor("xT", (K, M), f16, kind="ExternalInput").ap()
    qwq_ap = nc.dram_tensor("qwq", (QUADS, P, O_SHARD), u16, kind="ExternalInput").ap()
    szq_ap = nc.dram_tensor("szq", (QUADS, 4, 2, O_SHARD), f16, kind="ExternalInput").ap()
    bias_ap = nc.dram_tensor("bias", (1, O_SHARD), f16, kind="ExternalInput").ap()
    out_ap = nc.dram_tensor(
        "out", (n_mblocks * M_TILE, O_SHARD), f16, kind="ExternalOutput"
    ).ap()

    with tile.TileContext(nc) as tc:
      for _rep in range(repeat):
        with (
            tc.tile_pool(name="const", bufs=1) as const,
            tc.tile_pool(name="wt", bufs=4 * QUADS) as wt_pool,
            tc.tile_pool(name="deq", bufs=3) as deq,
            tc.tile_pool(name="xt", bufs=42) as xt_pool,
            tc.tile_pool(name="outp", bufs=6) as outp,
            tc.tile_pool(name="psum", bufs=8, space="PSUM") as psum,
        ):
            msk_sb = const.tile([P, 4], u32)
            for t in range(4):
                m = (0xF << (4 * t)) & 0xFFFF
                nc.gpsimd.memset(msk_sb[:, t : t + 1], (m << 16) | m)
            bias_rep = const.tile([P, O_SHARD], f16)
            nc.gpsimd.dma_start(
                out=bias_rep,
                in_=bass.AP(
                    tensor=bias_ap.tensor,
                    offset=bias_ap.offset,
                    ap=[[0, P], [1, O_SHARD]],
                ),
            )

            # per-chunk weight slices: WS[4q+t] = [128, 1376]
            WS = [
                wt_pool.tile([P, O_SHARD], f16, tag="wt", name=f"ws{_rep}_{kt}")
                for kt in range(G)
            ]

            # ---- dequant ----
            xts0 = []
            for q in range(QUADS):
                rt = deq.tile([P, O_SHARD], u16, tag="rt", bufs=6)
                nc.sync.dma_start(out=rt, in_=qwq_ap[q])
                s_b = deq.tile([P, O_SHARD], f16, tag="sb", bufs=6)
                nc.scalar.dma_start(
                    out=s_b,
                    in_=bass.AP(
                        tensor=szq_ap.tensor,
                        offset=szq_ap.offset + q * 4 * 2 * O_SHARD,
                        ap=[[2 * O_SHARD, 4], [0, 32], [1, O_SHARD]],
                    ),
                )
                zs_b = deq.tile([P, O_SHARD], f16, tag="zb", bufs=6)
                nc.scalar.dma_start(
                    out=zs_b,
                    in_=bass.AP(
                        tensor=szq_ap.tensor,
                        offset=szq_ap.offset + (q * 4 * 2 + 1) * O_SHARD,
                        ap=[[2 * O_SHARD, 4], [0, 32], [1, O_SHARD]],
                    ),
                )
                for t in range(4):
                    na = deq.tile([P, O_SHARD], u16, tag="na", bufs=3)
                    nc.vector.tensor_scalar(
                        out=na.bitcast(u32),
                        in0=rt.bitcast(u32),
                        scalar1=msk_sb[:, t : t + 1],
                        scalar2=None,
                        op0=mybir.AluOpType.bitwise_and,
                    )
                    nf = deq.tile([P, O_SHARD], f16, tag="nf", bufs=3)
                    nc.scalar.activation(
                        out=nf,
                        in_=na,
                        func=mybir.ActivationFunctionType.Copy,
                        scale=float(2.0 ** (-4 * t)),
                    )
                    wslice = WS[4 * q + t]
                    eng = nc.gpsimd if t == 3 else nc.vector
                    eng.tensor_tensor(
                        out=wslice,
                        in0=nf,
                        in1=s_b,
                        op=mybir.AluOpType.mult,
                    )
                    eng.tensor_tensor(
                        out=wslice,
                        in0=wslice,
                        in1=zs_b,
                        op=mybir.AluOpType.subtract,
                    )

            for g in range(G):
                xtile = xt_pool.tile([P, M_TILE], f16, tag="xt", name="xt")
                nc.sync.dma_start(
                    out=xtile, in_=x_ap[g * P : (g + 1) * P, 0:M_TILE]
                )
                xts0.append(xtile)

            # ---- matmul ----
            def finish_group(ps, mb, ms, o0, ow, qi=0):
                ot = outp.tile([P, 512], f16, tag="ot", name="ot")
                nc.vector.tensor_tensor(
                    out=ot[:, :ow],
                    in0=ps,
                    in1=bias_rep[:, o0 : o0 + ow],
                    op=mybir.AluOpType.add,
                )
                m0 = mb * M_TILE + ms * P
                qeng = nc.sync if (mb == n_mblocks - 1 and qi % 2) else nc.scalar
                qeng.dma_start(
                    out=out_ap[m0 : m0 + P, o0 : o0 + ow], in_=ot[:, :ow]
                )

            def kmajor_pass(mb, xts, groups):
                # one psum bank per (o-tile, ms); k-major so PE consumes
                # each dequant slice as soon as it lands
                pss = []
                for _ in groups:
                    pst = psum.tile([P, 512], f32, tag="ps", name="ps")
                    pss.append(pst)
                for kt in range(G):
                    for i, (o0, ow, ms) in enumerate(groups):
                        nc.tensor.matmul(
                            pss[i][:, :ow],
                            lhsT=xts[kt][:, ms * P : (ms + 1) * P],
                            rhs=WS[kt][:, o0 : o0 + ow],
                            start=(kt == 0),
                            stop=(kt == G - 1),
                        )
                for i, (o0, ow, ms) in enumerate(groups):
                    finish_group(pss[i][:, :ow], mb, ms, o0, ow)

            for mb in range(n_mblocks):
                if mb == 0:
                    xts = xts0
                else:
                    xts = []
                    for kt in range(G):
                        xtile = xt_pool.tile([P, M_TILE], f16, tag="xt", name="xt")
                        nc.sync.dma_start(
                            out=xtile,
                            in_=x_ap[
                                kt * P : (kt + 1) * P,
                                mb * M_TILE : (mb + 1) * M_TILE,
                            ],
                        )
                        xts.append(xtile)
                if mb == 0:
                    # dequant still streaming: pass A fills all 8 psum banks
                    # so the PE consumes each chunk as fast as it lands
                    kmajor_pass(0, xts, [(o0, ow, ms) for ms in (0, 1)
                                         for (o0, ow) in O_TILES]
                                        + [(0, 512, 2), (512, 512, 2)])
                    kmajor_pass(0, xts, [(1024, O_SHARD - 1024, 2)]
                                        + [(o0, ow, 3) for (o0, ow) in O_TILES])
                    continue
                for ms in range(M_TILE // P):
                    # 3 psum banks, kt-major, ob-inner: one LDWEIGHTS feeds
                    # the 3 o-tiles (redundant loads removed by ldw-opt)
                    pss = []
                    for _ in O_TILES:
                        pss.append(psum.tile([P, 512], f32, tag="ps", name="ps"))
                    for kt in range(G):
                        for i, (o0, ow) in enumerate(O_TILES):
                            nc.tensor.matmul(
                                pss[i][:, :ow],
                                lhsT=xts[kt][:, ms * P : (ms + 1) * P],
                                rhs=WS[kt][:, o0 : o0 + ow],
                                start=(kt == 0),
                                stop=(kt == G - 1),
                            )
                    for i, (o0, ow) in enumerate(O_TILES):
                        finish_group(pss[i][:, :ow], mb, ms, o0, ow, qi=i + ms)

    nc.compile()
    return nc


def _unpack_nib(a):
    shifts = (np.arange(8, dtype=np.int32) * 4).reshape(1, 1, 8)
    nib = (a[..., None] >> shifts) & 0xF
    return nib.reshape(a.shape[0], a.shape[1] * 8)


def make_in_maps(x, qweight, qzeros, scales, bias):
    # Chunk kt=(Q,t): partition p = j*32+c holds original k = (4Q+j)*128+4c+t.
    # Permute K accordingly and transpose to [K, M] so xT tiles load with
    # plain contiguous DMAs.
    x_flat = np.ascontiguousarray(
        x.reshape(M, QUADS, 4, 32, 4)      # [m, Q, j, c, t]
        .transpose(1, 4, 2, 3, 0)           # [Q, t, j, c, m]
        .reshape(K, M)
    )
    in_maps = []
    for i in range(N_CORES):
        sl = slice(i * O_SHARD, (i + 1) * O_SHARD)
        qw16 = np.ascontiguousarray(qweight[sl]).view(np.uint16)  # [O, 1024]
        # qwq[Q, j*32+c, o] = halfword (4Q+j)*32+c of row o
        qwq = np.ascontiguousarray(
            qw16.T.reshape(QUADS, 4 * 32, O_SHARD)
        )
        z = _unpack_nib(np.ascontiguousarray(qzeros[sl]))[:, :G].astype(np.float32)
        s = scales[sl, :G].astype(np.float32)
        zs = (z * s).astype(np.float16)  # [O, G]
        st = s.astype(np.float16)
        # szq[Q, g_in_quad, {s, zs}, o]
        szq = np.stack([st.T.reshape(G, O_SHARD), zs.T.reshape(G, O_SHARD)],
                       axis=1)               # [G, 2, O]
        szq = np.ascontiguousarray(szq.reshape(QUADS, 4, 2, O_SHARD))
        b = np.ascontiguousarray(bias[sl]).reshape(1, O_SHARD)
        in_maps.append(
            {"xT": x_flat, "qwq": qwq, "szq": szq, "bias": b}
        )
    return in_maps


_NC = None


def kernel(x, qweight, qzeros, scales, bias):
    global _NC
    x = np.asarray(x)
    qweight = np.asarray(qweight)
    qzeros = np.asarray(qzeros)
    scales = np.asarray(scales)
    bias = np.asarray(bias)
    if _NC is None:
        _NC = build()
    in_maps = make_in_maps(x, qweight, qzeros, scales, bias)
    res = bass_utils.run_bass_kernel_spmd(_NC, in_maps, core_ids=list(range(N_CORES)))
    shards = [res.results[i]["out"] for i in range(N_CORES)]
    out = np.concatenate(shards, axis=1).reshape(2, 2048, O_FULL)
    return out.astype(np.float16)


# revision 6
# speedup vs baseline: 1.1193x; 1.0006x over previous
"""AWQ 4-bit quantized linear (nn_AWQLinear) on 8 Trainium2 NeuronCores.

out[b,s,o] = fp16(sum_k x[b,s,k] * w[o,k]) + bias[o]
w[o,k] = (q[o,k] - z[o,k//128]) * s[o,k//128],  q packed 8 nibbles / int32.

Sharding: column-parallel (per spec hint). qweight/qzeros/scales/bias are
split along O=11008 into 8 shards of 1376; x is replicated; per-core
[4096, 1376] outputs are concatenated on host.

v4 layout — quad-packed dequant, per-slice weight tiles:
  K is processed in 32 chunks of 128; chunk kt = (Q, t) with Q = kt//4 a
  "quad" of 4 consecutive k-groups and t = kt%4 a nibble index. Partition
  p = j*32 + c of chunk (Q, t) holds original k = (4Q+j)*128 + 4c + t, so
  ONE [128, 1376] u16 tile ("qwq", halfword c of group 4Q+j at column o,
  host-gathered, unique bytes only) serves all four nibble extractions:
    slice t:  u32-bitcast AND with packed mask (DVE tensor_scalar, halves
              the column count; mask tiles built by memset, no DMA)
              -> ScalarE activation Copy(scale=2^-4t): u16 -> f16 nibble
              -> TT w = nib * s_b ; TT w -= zs_b  (f16, 2x tier;
                 slices 0-2 on DVE, slice 3 on gpsimd for balance)
  Each W slice is its OWN tile so a chunk's matmuls depend only on that
  slice's two writers (no whole-quad false dependency). s_b/zs_b arrive
  as separate per-quad DMAs that block-replicate 4 scale rows 32x across
  partitions ([4g,1376] -> [128,1376], 0.35 MB each, 6-deep pools so the
  x-tile stream can't starve them); zs = z*s host-prepped. x is
  host-permuted to the chunk layout and pre-transposed to [K, M].

Matmul: psum [m=128, o<=512] accumulates 32 k-chunk matmuls (lhsT = xT
tile slice, rhs = W chunk slice). mb0 runs k-major: pass A fills all 8
psum banks (ms0,1 x 3 o-tiles + ms2 x 2) consuming ~6.3us/quad to ride
the dequant wave; pass B covers the remaining 4 tiles. Later m-blocks
run ms-outer with 3 banks and ob-inner so consecutive matmuls share the
stationary operand. Epilogue: single DVE tensor_tensor adds the
partition-replicated bias while copying PSUM->SBUF fp16; outputs ride
the scalar HWDGE queue (last m-block alternates scalar/sync to halve
the final drain).
"""

import sys

sys.path.insert(0, "/opt/trn_rl_repo")

import numpy as np

import concourse.bass as bass
import concourse.tile as tile
from concourse import bacc, mybir
from concourse import bass_utils

P = 128
N_CORES = 8
O_FULL = 11008
O_SHARD = O_FULL // N_CORES  # 1376
K = 4096
G = 32  # k-groups of 128
QUADS = G // 4  # 8
M = 4096  # tokens = 2*2048
M_TILE = 512
O_TILES = [(0, 512), (512, 512), (1024, O_SHARD - 1024)]  # (offset, width)

f16 = mybir.dt.float16
u16 = mybir.dt.uint16
u32 = mybir.dt.uint32
f32 = mybir.dt.float32


def build(n_mblocks=M // M_TILE, repeat=1):
    nc = bacc.Bacc("TRN2", target_bir_lowering=False, debug=False, num_devices=N_CORES)

    x_ap = nc.dram_tensor("xT", (K, M), f16, kind="ExternalInput").ap()
    qwq_ap = nc.dram_tensor("qwq", (QUADS, P, O_SHARD), u16, kind="ExternalInput").ap()
    szq_ap = nc.dram_tensor("szq", (QUADS, 4, 2, O_SHARD), f16, kind="ExternalInput").ap()
    bias_ap = nc.dram_tensor("bias", (1, O_SHARD), f16, kind="ExternalInput").ap()
    out_ap = nc.dram_tensor(
        "out", (n_mblocks * M_TILE, O_SHARD), f16, kind="ExternalOutput"
    ).ap()

    with tile.TileContext(nc) as tc:
      for _rep in range(repeat):
        with (
            tc.tile_pool(name="const", bufs=1) as const,
            tc.tile_pool(name="wt", bufs=4 * QUADS) as wt_pool,
            tc.tile_pool(name="deq", bufs=3) as deq,
            tc.tile_pool(name="xt", bufs=42) as xt_pool,
            tc.tile_pool(name="outp", bufs=6) as outp,
            tc.tile_pool(name="psum", bufs=8, space="PSUM") as psum,
        ):
            msk_sb = const.tile([P, 4], u32)
            for t in range(4):
                m = (0xF << (4 * t)) & 0xFFFF
                nc.gpsimd.memset(msk_sb[:, t : t + 1], (m << 16) | m)
            bias_rep = const.tile([P, O_SHARD], f16)
            nc.gpsimd.dma_start(
                out=bias_rep,
                in_=bass.AP(
                    tensor=bias_ap.tensor,
                    offset=bias_ap.offset,
                    ap=[[0, P], [1, O_SHARD]],
                ),
            )

            # per-chunk weight slices: WS[4q+t] = [128, 1376]
            WS = [
                wt_pool.tile([P, O_SHARD], f16, tag="wt", name=f"ws{_rep}_{kt}")
                for kt in range(G)
            ]

            # ---- dequant ----
            xts0 = []
            for q in range(QUADS):
                rt = deq.tile([P, O_SHARD], u16, tag="rt", bufs=6)
                nc.sync.dma_start(out=rt, in_=qwq_ap[q])
                s_b = deq.tile([P, O_SHARD], f16, tag="sb", bufs=6)
                nc.sync.dma_start(
                    out=s_b,
                    in_=bass.AP(
                        tensor=szq_ap.tensor,
                        offset=szq_ap.offset + q * 4 * 2 * O_SHARD,
                        ap=[[2 * O_SHARD, 4], [0, 32], [1, O_SHARD]],
                    ),
                )
                zs_b = deq.tile([P, O_SHARD], f16, tag="zb", bufs=6)
                nc.sync.dma_start(
                    out=zs_b,
                    in_=bass.AP(
                        tensor=szq_ap.tensor,
                        offset=szq_ap.offset + (q * 4 * 2 + 1) * O_SHARD,
                        ap=[[2 * O_SHARD, 4], [0, 32], [1, O_SHARD]],
                    ),
                )
                for t in range(4):
                    na = deq.tile([P, O_SHARD], u16, tag="na", bufs=3)
                    nc.vector.tensor_scalar(
                        out=na.bitcast(u32),
                        in0=rt.bitcast(u32),
                        scalar1=msk_sb[:, t : t + 1],
                        scalar2=None,
                        op0=mybir.AluOpType.bitwise_and,
                    )
                    nf = deq.tile([P, O_SHARD], f16, tag="nf", bufs=3)
                    nc.scalar.activation(
                        out=nf,
                        in_=na,
                        func=mybir.ActivationFunctionType.Copy,
                        scale=float(2.0 ** (-4 * t)),
                    )
                    wslice = WS[4 * q + t]
                    eng = nc.gpsimd if (t == 3 and q < 6) else nc.vector
                    eng.tensor_tensor(
                        out=wslice,
                        in0=nf,
                        in1=s_b,
                        op=mybir.AluOpType.mult,
                    )
                    eng.tensor_tensor(
                        out=wslice,
                        in0=wslice,
                        in1=zs_b,
                        op=mybir.AluOpType.subtract,
                    )

            for g in range(G):
                xtile = xt_pool.tile([P, M_TILE], f16, tag="xt", name="xt")
                nc.sync.dma_start(
                    out=xtile, in_=x_ap[g * P : (g + 1) * P, 0:M_TILE]
                )
                xts0.append(xtile)

            # ---- matmul ----
            def finish_group(ps, mb, ms, o0, ow, qi=0):
                ot = outp.tile([P, 512], f16, tag="ot", name="ot")
                nc.vector.tensor_tensor(
                    out=ot[:, :ow],
                    in0=ps,
                    in1=bias_rep[:, o0 : o0 + ow],
                    op=mybir.AluOpType.add,
                )
                m0 = mb * M_TILE + ms * P
                qeng = nc.sync if qi % 2 else nc.scalar
                qeng.dma_start(
                    out=out_ap[m0 : m0 + P, o0 : o0 + ow], in_=ot[:, :ow]
                )

            def kmajor_pass(mb, xts, groups):
                # one psum bank per (o-tile, ms); k-major so PE consumes
                # each dequant slice as soon as it lands
                pss = []
                for _ in groups:
                    pst = psum.tile([P, 512], f32, tag="ps", name="ps")
                    pss.append(pst)
                for kt in range(G):
                    for i, (o0, ow, ms) in enumerate(groups):
                        nc.tensor.matmul(
                            pss[i][:, :ow],
                            lhsT=xts[kt][:, ms * P : (ms + 1) * P],
                            rhs=WS[kt][:, o0 : o0 + ow],
                            start=(kt == 0),
                            stop=(kt == G - 1),
                        )
                for i, (o0, ow, ms) in enumerate(groups):
                    finish_group(pss[i][:, :ow], mb, ms, o0, ow, qi=i)

            for mb in range(n_mblocks):
                if mb == 0:
                    xts = xts0
                else:
                    xts = []
                    for kt in range(G):
                        xtile = xt_pool.tile([P, M_TILE], f16, tag="xt", name="xt")
                        nc.sync.dma_start(
                            out=xtile,
                            in_=x_ap[
                                kt * P : (kt + 1) * P,
                                mb * M_TILE : (mb + 1) * M_TILE,
                            ],
                        )
                        xts.append(xtile)
                if mb == 0:
                    # dequant still streaming: pass A fills all 8 psum banks
                    # so the PE consumes each chunk as fast as it lands
                    kmajor_pass(0, xts, [(o0, ow, ms) for ms in (0, 1)
                                         for (o0, ow) in O_TILES]
                                        + [(0, 512, 2), (512, 512, 2)])
                    kmajor_pass(0, xts, [(1024, O_SHARD - 1024, 2)]
                                        + [(o0, ow, 3) for (o0, ow) in O_TILES])
                    continue
                for ms in range(M_TILE // P):
                    # 3 psum banks, kt-major, ob-inner: one LDWEIGHTS feeds
                    # the 3 o-tiles (redundant loads removed by ldw-opt)
                    pss = []
                    for _ in O_TILES:
                        pss.append(psum.tile([P, 512], f32, tag="ps", name="ps"))
                    for kt in range(G):
                        for i, (o0, ow) in enumerate(O_TILES):
                            nc.tensor.matmul(
                                pss[i][:, :ow],
                                lhsT=xts[kt][:, ms * P : (ms + 1) * P],
                                rhs=WS[kt][:, o0 : o0 + ow],
                                start=(kt == 0),
                                stop=(kt == G - 1),
                            )
                    for i, (o0, ow) in enumerate(O_TILES):
                        finish_group(pss[i][:, :ow], mb, ms, o0, ow, qi=i + ms)

    nc.compile()
    return nc


def _unpack_nib(a):
    shifts = (np.arange(8, dtype=np.int32) * 4).reshape(1, 1, 8)
    nib = (a[..., None] >> shifts) & 0xF
    return nib.reshape(a.shape[0], a.shape[1] * 8)


def make_in_maps(x, qweight, qzeros, scales, bias):
    # Chunk kt=(Q,t): partition p = j*32+c holds original k = (4Q+j)*128+4c+t.
    # Permute K accordingly and transpose to [K, M] so xT tiles load with
    # plain contiguous DMAs.
    x_flat = np.ascontiguousarray(
        x.reshape(M, QUADS, 4, 32, 4)      # [m, Q, j, c, t]
        .transpose(1, 4, 2, 3, 0)           # [Q, t, j, c, m]
        .reshape(K, M)
    )
    in_maps = []
    for i in range(N_CORES):
        sl = slice(i * O_SHARD, (i + 1) * O_SHARD)
        qw16 = np.ascontiguousarray(qweight[sl]).view(np.uint16)  # [O, 1024]
        # qwq[Q, j*32+c, o] = halfword (4Q+j)*32+c of row o
        qwq = np.ascontiguousarray(
            qw16.T.reshape(QUADS, 4 * 32, O_SHARD)
        )
        z = _unpack_nib(np.ascontiguousarray(qzeros[sl]))[:, :G].astype(np.float32)
        s = scales[sl, :G].astype(np.float32)
        zs = (z * s).astype(np.float16)  # [O, G]
        st = s.astype(np.float16)
        # szq[Q, g_in_quad, {s, zs}, o]
        szq = np.stack([st.T.reshape(G, O_SHARD), zs.T.reshape(G, O_SHARD)],
                       axis=1)               # [G, 2, O]
        szq = np.ascontiguousarray(szq.reshape(QUADS, 4, 2, O_SHARD))
        b = np.ascontiguousarray(bias[sl]).reshape(1, O_SHARD)
        in_maps.append(
            {"xT": x_flat, "qwq": qwq, "szq": szq, "bias": b}
        )
    return in_maps


_NC = None


def kernel(x, qweight, qzeros, scales, bias):
    global _NC
    x = np.asarray(x)
    qweight = np.asarray(qweight)
    qzeros = np.asarray(qzeros)
    scales = np.asarray(scales)
    bias = np.asarray(bias)
    if _NC is None:
        _NC = build()
    in_maps = make_in_maps(x, qweight, qzeros, scales, bias)
    res = bass_utils.run_bass_kernel_spmd(_NC, in_maps, core_ids=list(range(N_CORES)))
    shards = [res.results[i]["out"] for i in range(N_CORES)]
    out = np.concatenate(shards, axis=1).reshape(2, 2048, O_FULL)
    return out.astype(np.float16)
